# revision 1
# baseline (speedup 1.0000x reference)
"""CrossAttention Trainium2 kernel.

Reference computation (per batch b):
  q = x @ Wq; k = ctx @ Wk; v = ctx @ Wv   (multi-head, H=8, DH=64)
  out = softmax(q k^T / sqrt(DH)) v @ Wo + bo, rows >= seq_len zeroed.

Strategy: only rows < seq_len are computed ("ragged"); valid query tiles
(512 rows) are distributed across the 8 cores with a statically uniform
per-core structure: every core runs CAP query tiles, the first CAP_A of
which read KV slot A and the rest KV slot B. Which batch each slot holds
is per-core DATA (context tensors), so one SPMD program serves all cores.

On-chip layout is fully "transposed" (sequence on the free dim) so no
on-chip transposes are needed:
  xT [DQ, Lt] -> qT = Wq^T xT [INNER, Lt]
  kT = Wk^T ctxT [INNER, S];  v_aug = (ctx_aug @ Wv_aug) [S, 8*(DH+1)]
  scoresT_h [S, Lt] = kT_h^T qT_h  (per head, K=DH)
  expT = exp(scoresT)  (no max subtraction needed: logits ~ N(0,1))
  avT_h [DH+1, Lt] = v_aug_h^T expT_h  (extra ones column -> row DH = colsum)
  normalize per head via reciprocal + partition-broadcast + multiply
  out [Lt, DQ] = outT_aug^T @ Wo_aug (ones K-row adds bo)
Matmuls run in float32r (full-rate fp32, ~1e-4 rel err).
"""

import math
import sys

sys.path.insert(0, "/opt/trn_rl_repo")

import numpy as np

B, L, S = 8, 8192, 512
DQ, DC = 256, 768
H, DH = 8, 64
INNER = H * DH
TL = 512          # query rows per tile
N_CORES = 8
DCA = DC + 1      # ctx augmented with a ones row
WVN = H * (DH + 1)  # 520: v augmented with a ones column per head


def _plan(nt):
    """Choose (CAP_A, CAP_B) and per-core pieces. Returns
    (cap_a, cap_b, cores) where cores is a list of 8 entries
    [(batch_a, tile0_a), (batch_b, tile0_b)] (batch -1 = padding)."""
    best = None
    lo = max(1, math.ceil(sum(nt) / N_CORES))
    for cap in range(lo, max(max(nt), lo) + 9):
        # NB=1: whole batches in CAP-size pieces
        if sum(math.ceil(n / cap) for n in nt) <= N_CORES:
            cost = cap * 18 + 12
            if best is None or cost < best[0]:
                best = (cost, cap, 0, None)
        # NB=2 split
        for a in range(cap - 1, 0, -1):
            b = cap - a
            opts = []
            for n in nt:
                o = []
                for ka in range(0, N_CORES + 1):
                    rem = n - a * ka
                    kb = max(0, math.ceil(rem / b))
                    if kb <= N_CORES:
                        o.append((ka, kb))
                opts.append(o)

            found = None

            def dfs(i, ta, tb, acc):
                nonlocal found
                if found is not None:
                    return
                if i == len(opts):
                    found = list(acc)
                    return
                for ka, kb in opts[i]:
                    if ta + ka <= N_CORES and tb + kb <= N_CORES:
                        acc.append((ka, kb))
                        dfs(i + 1, ta + ka, tb + kb, acc)
                        acc.pop()
                        if found is not None:
                            return

            dfs(0, 0, 0, [])
            if found is not None:
                cost = cap * 18 + 24
                if best is None or cost < best[0]:
                    best = (cost, cap, 1, (a, b, found))
                break  # larger a preferred; next a adds nothing
    assert best is not None
    _, cap, kind, info = best
    if kind == 0:
        # NB=1: emit as (a=cap, b=0-like) with slot B duplicating slot A
        pieces_a = []
        for bi, n in enumerate(nt):
            for j in range(math.ceil(n / cap)):
                pieces_a.append((bi, j * cap))
        while len(pieces_a) < N_CORES:
            pieces_a.append((-1, 0))
        cores = [[pa, (-1, 0)] for pa in pieces_a]
        return cap, 0, cores
    a, bsz, ks = info
    pieces_a, pieces_b = [], []
    for bi, n in enumerate(nt):
        ka, kb = ks[bi]
        t = 0
        for _ in range(ka):
            pieces_a.append((bi, t))
            t += a
        for _ in range(kb):
            pieces_b.append((bi, t))
            t += bsz
    while len(pieces_a) < N_CORES:
        pieces_a.append((-1, 0))
    while len(pieces_b) < N_CORES:
        pieces_b.append((-1, 0))
    cores = [[pieces_a[i], pieces_b[i]] for i in range(N_CORES)]
    return a, bsz, cores


_PROG_CACHE = {}


def _build_program(cap_a, cap_b):
    import concourse.mybir as mybir
    import concourse.tile as tile
    from concourse import bacc

    f32 = mybir.dt.float32
    f32r = mybir.dt.float32r
    CAP = cap_a + cap_b
    NSLOT = 2 if cap_b > 0 else 1

    nc = bacc.Bacc("TRN2", target_bir_lowering=False, debug=False,
                   num_devices=N_CORES)
    xT = nc.declare_dram_parameter("xT", [DQ, CAP * TL], f32r, isOutput=False)
    ctxs = [nc.declare_dram_parameter(f"ctx{s}", [DC, S], f32r, isOutput=False)
            for s in range(NSLOT)]
    wq = nc.declare_dram_parameter("wq", [DQ, INNER], f32r, isOutput=False)
    wk = nc.declare_dram_parameter("wk", [DC, INNER], f32r, isOutput=False)
    wv = nc.declare_dram_parameter("wv", [DC, INNER], f32r, isOutput=False)
    wo = nc.declare_dram_parameter("wo", [INNER, DQ], f32r, isOutput=False)
    bob = nc.declare_dram_parameter("bob", [128, DQ], f32, isOutput=False)
    vones = nc.declare_dram_parameter("vones", [128, 8, 1], f32r, isOutput=False)
    y = nc.declare_dram_parameter("y", [CAP * TL, DQ], f32, isOutput=True)

    with tile.TileContext(nc) as tc:
        with (
            tc.tile_pool(name="wpool", bufs=1) as wpool,
            tc.tile_pool(name="kvpool", bufs=1) as kvpool,
            tc.tile_pool(name="ctxpool", bufs=1) as ctxpool,
            tc.tile_pool(name="mpool", bufs=4) as mpool,
            tc.tile_pool(name="qpool", bufs=3) as qpool,
            tc.tile_pool(name="epool", bufs=5) as epool,
            tc.tile_pool(name="opool", bufs=2) as opool,
            tc.tile_pool(name="spool", bufs=4) as spool,
            tc.tile_pool(name="ypool", bufs=4) as ypool,
            tc.tile_pool(name="ps_big", bufs=2, space="PSUM") as ps_big,
            tc.tile_pool(name="ps_sc", bufs=2, space="PSUM") as ps_sc,
            tc.tile_pool(name="ps_av", bufs=2, space="PSUM") as ps_av,
        ):
            # ---- load weights; DMA order tracks first-use order:
            # wk+ctx0 (KV slot A) -> wq+x0/x1 (tile-0 qT) -> rest
            wk_sb = [wpool.tile([128, INNER], f32r, tag=f"wk{i}", name=f"wk{i}") for i in range(6)]
            for i in range(6):
                nc.sync.dma_start(wk_sb[i][:], wk[i * 128:(i + 1) * 128, :])
            ctx_tiles = {}
            for s in range(NSLOT):
                ctx_tiles[s] = [ctxpool.tile([128, S], f32r, tag=f"ctx{s}_{i}", name=f"ctx{s}_{i}")
                                for i in range(6)]
            for i in range(6):
                nc.sync.dma_start(ctx_tiles[0][i][:], ctxs[0][i * 128:(i + 1) * 128, :])
            wq_sb = [wpool.tile([128, INNER], f32r, tag=f"wq{i}", name=f"wq{i}") for i in range(2)]
            for i in range(2):
                nc.sync.dma_start(wq_sb[i][:], wq[i * 128:(i + 1) * 128, :])
            pre_x = {}
            for t in range(min(2, CAP)):
                xt_t = [mpool.tile([128, TL], f32r, tag=f"x{kc}", name=f"x{kc}")
                        for kc in range(2)]
                for kc in range(2):
                    nc.sync.dma_start(
                        xt_t[kc][:], xT[kc * 128:(kc + 1) * 128, t * TL:(t + 1) * TL])
                pre_x[t] = xt_t
            wv_sb = [wpool.tile([128, INNER], f32r, tag=f"wv{i}", name=f"wv{i}") for i in range(6)]
            for i in range(6):
                nc.sync.dma_start(wv_sb[i][:], wv[i * 128:(i + 1) * 128, :])
            for s in range(1, NSLOT):
                for i in range(6):
                    nc.sync.dma_start(ctx_tiles[s][i][:], ctxs[s][i * 128:(i + 1) * 128, :])
            wo_sb = [wpool.tile([128, DQ], f32r, tag=f"wo{i}", name=f"wo{i}") for i in range(4)]
            for i in range(4):
                nc.sync.dma_start(wo_sb[i][:], wo[i * 128:(i + 1) * 128, :])
            bob_sb = wpool.tile([128, DQ], f32, tag="bob", name="bob")
            nc.sync.dma_start(bob_sb[:], bob[:])

            # ---- KV phase per slot (emitted lazily before its tile group,
            # so slot-B KV doesn't block slot-A tiles on the in-order PE) ----
            kT = {}
            vA = {}
            def kv_phase(s):
                    ctx_sb = ctx_tiles[s]

                    kT[s] = [kvpool.tile([128, S], f32r, tag=f"kT{s}_{m}", name=f"kT{s}_{m}")
                             for m in range(4)]
                    for m in range(4):
                        pk = ps_big.tile([128, S], f32, tag="big", name="big")
                        for kc in range(6):
                            nc.tensor.matmul(
                                pk[:], wk_sb[kc][:, m * 128:(m + 1) * 128],
                                ctx_sb[kc][:], start=(kc == 0), stop=(kc == 5))
                        nc.vector.tensor_copy(kT[s][m][:], pk[:])
                    vA[s] = [kvpool.tile([128, WVN], f32r, tag=f"v{s}_{sc}", name=f"v{s}_{sc}")
                             for sc in range(4)]
                    for sc in range(4):
                        pv = ps_big.tile([128, 512], f32, tag="big", name="big")
                        for kc in range(6):
                            nc.tensor.matmul(
                                pv[:],
                                ctx_sb[kc][:, sc * 128:(sc + 1) * 128],
                                wv_sb[kc][:], start=(kc == 0), stop=(kc == 5))
                        vdst = vA[s][sc][:].rearrange("p (h d) -> p h d", d=DH + 1)
                        nc.vector.tensor_copy(
                            vdst[:, :, 0:DH],
                            pv[:].rearrange("p (h d) -> p h d", d=DH))
                        nc.sync.dma_start(vdst[:, :, DH:DH + 1], vones[:])

            # ---- main loop over query tiles ----
            pending_oproj = []

            def emit_oproj(tt, outT_t):
                for lsub in range(4):
                    po = ps_av.tile([128, DQ], f32, tag="av", name="av")
                    for kc in range(4):
                        nc.tensor.matmul(
                            po[:], outT_t[kc][:, lsub * 128:(lsub + 1) * 128],
                            wo_sb[kc][:], start=(kc == 0), stop=(kc == 3))
                    yt = ypool.tile([128, DQ], f32, tag="y", name="y")
                    nc.vector.tensor_add(yt[:], po[:], bob_sb[:])
                    nc.sync.dma_start(
                        y[tt * TL + lsub * 128: tt * TL + (lsub + 1) * 128, :],
                        yt[:])

            for t in range(CAP):
                s = 0 if t < cap_a else 1
                if t == 0:
                    kv_phase(0)
                if t == cap_a and NSLOT > 1:
                    kv_phase(1)
                elif t == 0 and NSLOT > 1 and cap_a == 0:
                    kv_phase(1)
                if t in pre_x:
                    xt = pre_x.pop(t)
                else:
                    xt = [mpool.tile([128, TL], f32r, tag=f"x{kc}", name=f"x{kc}") for kc in range(2)]
                    for kc in range(2):
                        nc.sync.dma_start(
                            xt[kc][:], xT[kc * 128:(kc + 1) * 128, t * TL:(t + 1) * TL])
                qT = [qpool.tile([128, TL], f32r, tag=f"q{m}", name=f"q{m}") for m in range(4)]
                for m in range(4):
                    pq = ps_big.tile([128, TL], f32, tag="big", name="big")
                    for kc in range(2):
                        nc.tensor.matmul(
                            pq[:], wq_sb[kc][:, m * 128:(m + 1) * 128],
                            xt[kc][:], start=(kc == 0), stop=(kc == 1))
                    nc.vector.tensor_copy(qT[m][:], pq[:])

                outT = [opool.tile([128, TL], f32r, tag=f"o{m}", name=f"o{m}") for m in range(4)]

                def emit_scores(h):
                    c, half = h // 2, h % 2
                    expT = []
                    for g in range(2):
                        psc = ps_sc.tile([128, 2, TL], f32, tag="sc", name="sc")
                        for j in range(2):
                            sc = g * 2 + j
                            nc.tensor.matmul(
                                psc[:, j, :],
                                kT[s][c][half * 64:(half + 1) * 64,
                                         sc * 128:(sc + 1) * 128],
                                qT[c][half * 64:(half + 1) * 64, :],
                                start=True, stop=True)
                        e = epool.tile([128, 2, TL], f32r, tag=f"e{g}", name=f"e{g}")
                        nc.scalar.activation(
                            e[:], psc[:], mybir.ActivationFunctionType.Exp)
                        expT.extend([e[:, 0, :], e[:, 1, :]])
                    return expT

                # 1-head lookahead: emit next head's scores before this head's
                # AV so the in-order PE never stalls ACT
                pend = emit_scores(0)
                while pending_oproj:
                    emit_oproj(*pending_oproj.pop(0))
                for h in range(H):
                    c, half = h // 2, h % 2
                    expT = pend
                    if h + 1 < H:
                        pend = emit_scores(h + 1)
                    pav = ps_av.tile([DH + 1, TL], f32, tag="av", name="av")
                    for sc in range(4):
                        nc.tensor.matmul(
                            pav[:], vA[s][sc][:, h * (DH + 1):(h + 1) * (DH + 1)],
                            expT[sc], start=(sc == 0), stop=(sc == 3))
                    rp = spool.tile([1, TL], f32, tag="rp", name="rp")
                    nc.vector.reciprocal(rp[:], pav[DH:DH + 1, :])
                    bc = spool.tile([64, TL], f32, tag="bc", name="bc")
                    nc.gpsimd.partition_broadcast(bc[:], rp[0:1, :])
                    nc.vector.tensor_mul(
                        outT[c][half * 64:(half + 1) * 64, :],
                        pav[0:DH, :], bc[:])

                pending_oproj.append((t, outT))
            while pending_oproj:
                emit_oproj(*pending_oproj.pop(0))
    nc.compile()
    return nc


def kernel(x, context, seq_lens, Wq, Wk, Wv, Wo, bo):
    from concourse.bass_utils import run_bass_kernel_spmd

    x = np.asarray(x, dtype=np.float32)
    context = np.asarray(context, dtype=np.float32)
    seq_lens = np.asarray(seq_lens, dtype=np.int32)
    Wq = np.asarray(Wq, dtype=np.float32)
    Wk = np.asarray(Wk, dtype=np.float32)
    Wv = np.asarray(Wv, dtype=np.float32)
    Wo = np.asarray(Wo, dtype=np.float32)
    bo = np.asarray(bo, dtype=np.float32)

    lens = np.clip(seq_lens, 1, L)
    nt = [int(math.ceil(int(n) / TL)) for n in lens]
    cap_a, cap_b, cores = _plan(nt)
    CAP = cap_a + cap_b
    NSLOT = 2 if cap_b > 0 else 1

    key = (cap_a, cap_b)
    if key not in _PROG_CACHE:
        _PROG_CACHE[key] = _build_program(cap_a, cap_b)
    nc = _PROG_CACHE[key]

    # shared (replicated) weights
    scale = 1.0 / math.sqrt(DH)
    wq_in = (Wq * scale).astype(np.float32)
    wv_in = np.ascontiguousarray(Wv.astype(np.float32))
    wo_in = np.ascontiguousarray(Wo.astype(np.float32))
    bob_in = np.broadcast_to(bo[None, :], (128, DQ)).copy()
    vones_in = np.ones((128, 8, 1), dtype=np.float32)

    in_maps = []
    for core in range(N_CORES):
        xt_core = np.zeros((CAP * TL, DQ), dtype=np.float32)
        m = {}
        for sidx in range(NSLOT):
            bi, t0 = cores[core][sidx]
            npieces = cap_a if sidx == 0 else cap_b
            if bi >= 0:
                r0 = t0 * TL
                r1 = min(r0 + npieces * TL, L)
                if r1 > r0:
                    off = sidx * cap_a * TL
                    xt_core[off:off + (r1 - r0)] = x[bi, r0:r1]
                cb = context[bi]
            else:
                cb = context[0]
            m[f"ctx{sidx}"] = np.ascontiguousarray(cb.T)
        m["xT"] = np.ascontiguousarray(xt_core.T)
        m["wq"] = wq_in
        m["wk"] = Wk
        m["wv"] = wv_in
        m["wo"] = wo_in
        m["bob"] = bob_in
        m["vones"] = vones_in
        in_maps.append(m)

    res = run_bass_kernel_spmd(nc, in_maps, list(range(N_CORES)))

    out = np.zeros((B, L, DQ), dtype=np.float32)
    for core in range(N_CORES):
        yc = res.results[core]["y"]
        for sidx in range(NSLOT):
            bi, t0 = cores[core][sidx]
            if bi < 0:
                continue
            npieces = cap_a if sidx == 0 else cap_b
            r0 = t0 * TL
            r1 = min(r0 + npieces * TL, int(lens[bi]))
            if r1 > r0:
                off = sidx * cap_a * TL
                out[bi, r0:r1] = yc[off:off + (r1 - r0)]
    return out



# revision 9
# speedup vs baseline: 1.0817x; 1.0817x over previous
"""CrossAttention Trainium2 kernel (bf16 matmuls, multi-slot ragged plan).

Reference computation (per batch b):
  q = x @ Wq; k = ctx @ Wk; v = ctx @ Wv   (multi-head, H=8, DH=64)
  out = softmax(q k^T / sqrt(DH)) v @ Wo + bo, rows >= seq_len zeroed.

Strategy: only rows < seq_len are computed ("ragged"); valid query tiles
(512 rows) are distributed across the 8 cores with a statically uniform
per-core structure: every core runs CAP = sum(caps) query tiles split into
up to 3 KV "slots" of caps[j] tiles each; slot j reads its own context
tensor. Which batch each slot holds is per-core DATA, so one SPMD program
serves all cores.

All matmuls run in bf16 (full PE rate; fp8 DoubleRow was tried and is 2x
faster on paper, but fp8's ~4% value noise blows the max-norm accuracy
gate on queries whose softmax concentrates on a single key).

On-chip layout is fully "transposed" (sequence on the free dim):
  xT [DQ, Lt] -> qT = Wq^T xT [INNER, Lt]
  kT = Wk^T ctxT [INNER, S];  v_aug = (ctx @ Wv | 1) per head [S, 8*65]
  scoresT_h [S, Lt] = kT_h^T qT_h  (per head, K=DH)
  expT = exp(scoresT) -> bf16  (no max subtraction: logits ~ N(0,1))
  pav [65, Lt] = v_aug_h^T expT_h  (ones column -> row 64 = denominator)
  per head: DVE recip -> Pool partition_broadcast -> DVE mul -> outT bf16
  out [Lt, DQ] = outT^T @ Wo + bo (bias added on DVE), y f32.

Emission spreads qproj/oproj/KV-projection work units across the head
loop so the in-order PE never clumps and starves the other engines; each
tensor is loaded with one merged DMA (the issuing sequencer serializes
DMA issues at ~650ns each), with the first KV/ctx loads issued on the
otherwise-idle Activation queue to shorten the prologue.
"""

import math
import sys

sys.path.insert(0, "/opt/trn_rl_repo")

import numpy as np
import ml_dtypes

B, L, S = 8, 8192, 512
DQ, DC = 256, 768
H, DH = 8, 64
INNER = H * DH
TL = 512          # query rows per tile
N_CORES = 8
VW = DH + 1       # v columns per head (64 v + ones)
WVN = H * VW      # 520


def _try_assign(nt, caps):
    """Per batch, pick piece counts k_j (caps[j]-sized pieces) covering
    nt[b] tiles, with per-slot totals <= N_CORES."""
    nslot = len(caps)
    opts = []
    for n in nt:
        o = []
        def gen(j, ks, cov):
            if j == nslot:
                if cov >= n:
                    o.append((sum(ks), tuple(ks)))
                return
            for k in range(0, N_CORES + 1):
                c = cov + k * caps[j]
                if c > n + caps[0] and k > 0:
                    break
                gen(j + 1, ks + [k], c)
        gen(0, [], 0)
        if not o:
            return None
        o.sort()
        opts.append([ks for _, ks in o])

    found = None
    used_acc = []

    def dfs(i, used):
        nonlocal found
        if found is not None:
            return
        if i == len(opts):
            found = list(used_acc)
            return
        for ks in opts[i]:
            nu = tuple(u + k for u, k in zip(used, ks))
            if all(u <= N_CORES for u in nu):
                used_acc.append(ks)
                dfs(i + 1, nu)
                used_acc.pop()
                if found is not None:
                    return

    dfs(0, tuple([0] * nslot))
    return found


def _plan(nt):
    """Choose caps (1-3 slots) and per-core piece placement. Returns
    (caps, cores) with cores[i][j] = (batch, tile0), batch -1 = pad."""
    best = None
    for cap_total in range(max(1, math.ceil(sum(nt) / N_CORES)),
                           max(nt) + 3):
        cands = []
        for c0 in range(cap_total, 0, -1):
            r = cap_total - c0
            if r == 0:
                cands.append((c0,))
                continue
            for c1 in range(min(c0, r), 0, -1):
                r2 = r - c1
                if r2 == 0:
                    cands.append((c0, c1))
                elif r2 <= c1:
                    cands.append((c0, c1, r2))
        for caps in sorted(cands, key=len):
            asg = _try_assign(nt, caps)
            if asg is not None:
                best = (caps, asg)
                break
        if best is not None:
            break
    assert best is not None
    caps, asg = best
    nslot = len(caps)
    pieces = [[] for _ in range(nslot)]
    for bi, ks in enumerate(asg):
        t0 = 0
        for j in range(nslot):
            for _ in range(ks[j]):
                pieces[j].append((bi, t0))
                t0 += caps[j]
    cores = []
    for i in range(N_CORES):
        cores.append([pieces[j][i] if i < len(pieces[j]) else (-1, 0)
                      for j in range(nslot)])
    return caps, cores


_PROG_CACHE = {}


def _build_program(caps):
    import concourse.mybir as mybir
    import concourse.tile as tile
    from concourse import bacc

    f32 = mybir.dt.float32
    bf16 = mybir.dt.bfloat16
    CAP = sum(caps)
    NSLOT = len(caps)
    starts = [sum(caps[:j]) for j in range(NSLOT)]

    nc = bacc.Bacc("TRN2", target_bir_lowering=False, debug=False,
                   num_devices=N_CORES)
    xT = nc.declare_dram_parameter("xT", [DQ, CAP * TL], bf16, isOutput=False)
    ctxs = [nc.declare_dram_parameter(f"ctx{j}", [DC, S], bf16,
                                      isOutput=False) for j in range(NSLOT)]
    wq = nc.declare_dram_parameter("wq", [DQ, INNER], bf16, isOutput=False)
    wk = nc.declare_dram_parameter("wk", [DC, INNER], bf16, isOutput=False)
    wv = nc.declare_dram_parameter("wv", [DC, INNER], bf16, isOutput=False)
    wo = nc.declare_dram_parameter("wo", [INNER, DQ], bf16, isOutput=False)
    bob = nc.declare_dram_parameter("bob", [128, DQ], f32, isOutput=False)
    vones = nc.declare_dram_parameter("vones", [128, 8, 1], bf16,
                                      isOutput=False)
    y = nc.declare_dram_parameter("y", [CAP * TL, DQ], f32, isOutput=True)

    with tile.TileContext(nc) as tc:
        with (
            tc.tile_pool(name="wpool", bufs=1) as wpool,
            tc.tile_pool(name="kvpool", bufs=1) as kvpool,
            tc.tile_pool(name="ctxpool", bufs=2) as ctxpool,
            tc.tile_pool(name="mpool", bufs=3) as mpool,
            tc.tile_pool(name="qpool", bufs=2) as qpool,
            tc.tile_pool(name="epool", bufs=5) as epool,
            tc.tile_pool(name="opool", bufs=2) as opool,
            tc.tile_pool(name="spool", bufs=6) as spool,
            tc.tile_pool(name="ypool", bufs=4) as ypool,
            tc.tile_pool(name="ps_big", bufs=2, space="PSUM") as ps_big,
            tc.tile_pool(name="ps_sc", bufs=2, space="PSUM") as ps_sc,
            tc.tile_pool(name="ps_av", bufs=2, space="PSUM") as ps_av,
        ):
            # ---- prologue loads; one merged DMA per tensor. ctx0/wk go on
            # the idle ACT queue so kproj can start ~2us in; the rest on SP.
            ctx_tiles = {}

            def load_ctx(j, eng=None):
                t = ctxpool.tile([128, 6, S], bf16, tag="ctx", name=f"ctx{j}")
                (eng or nc.sync).dma_start(
                    t[:], ctxs[j][:].rearrange("(kc p) s -> p kc s", p=128))
                ctx_tiles[j] = t

            load_ctx(0)
            wk_sb = wpool.tile([128, 6, INNER], bf16, tag="wk", name="wk")
            nc.scalar.dma_start(
                wk_sb[:], wk[:].rearrange("(kc p) c -> p kc c", p=128))
            wq_sb = wpool.tile([128, 2, INNER], bf16, tag="wq", name="wq")
            nc.sync.dma_start(
                wq_sb[:], wq[:].rearrange("(kc p) c -> p kc c", p=128))
            pre_x = {}

            def load_x(t):
                xt = mpool.tile([128, 2, TL], bf16, tag="x", name=f"x{t}")
                nc.sync.dma_start(
                    xt[:], xT[:, t * TL:(t + 1) * TL]
                    .rearrange("(kc p) t -> p kc t", p=128))
                pre_x[t] = xt

            load_x(0)
            wv_sb = wpool.tile([128, 6, INNER], bf16, tag="wv", name="wv")
            nc.scalar.dma_start(
                wv_sb[:], wv[:].rearrange("(kc p) c -> p kc c", p=128))
            if CAP > 1:
                load_x(1)
            wo_sb = wpool.tile([128, 4, DQ], bf16, tag="wo", name="wo")
            nc.sync.dma_start(
                wo_sb[:], wo[:].rearrange("(kc p) d -> p kc d", p=128))
            bob_sb = wpool.tile([128, DQ], f32, tag="bob", name="bob")
            nc.sync.dma_start(bob_sb[:], bob[:])

            kT = {}
            vA = {}

            def kv_alloc(j):
                kT[j] = [kvpool.tile([128, S], bf16, tag=f"kT{j}_{m}",
                                     name=f"kT{j}_{m}") for m in range(4)]
                vA[j] = [kvpool.tile([128, WVN], bf16, tag=f"v{j}_{sc}",
                                     name=f"v{j}_{sc}") for sc in range(4)]

            def kproj(j, m):
                ctx_sb = ctx_tiles[j]
                pk = ps_big.tile([128, S], f32, tag="big", name="big")
                for kc in range(6):
                    nc.tensor.matmul(
                        pk[:], wk_sb[:, kc, m * 128:(m + 1) * 128],
                        ctx_sb[:, kc, :], start=(kc == 0), stop=(kc == 5))
                nc.vector.tensor_copy(kT[j][m][:], pk[:])

            def vproj(j, sc):
                ctx_sb = ctx_tiles[j]
                pv = ps_big.tile([128, INNER], f32, tag="big", name="big")
                for kc in range(6):
                    nc.tensor.matmul(
                        pv[:], ctx_sb[:, kc, sc * 128:(sc + 1) * 128],
                        wv_sb[:, kc, :], start=(kc == 0), stop=(kc == 5))
                vdst = vA[j][sc][:].rearrange("p (h d) -> p h d", d=VW)
                nc.vector.tensor_copy(
                    vdst[:, :, 0:DH],
                    pv[:].rearrange("p (h d) -> p h d", d=DH))
                nc.sync.dma_start(vdst[:, :, DH:DH + 1], vones[:])

            def kv_units(j):
                kv_alloc(j)
                units = [lambda m=m: kproj(j, m) for m in range(4)]
                units += [lambda sc=sc: vproj(j, sc) for sc in range(4)]
                if j + 1 < NSLOT:
                    units.append(lambda jj=j + 1: load_ctx(jj))
                return units

            qT = {}

            def qproj_unit(t, m):
                if m == 0:
                    qT[t] = [qpool.tile([128, TL], bf16, tag=f"q{mm}",
                                        name=f"q{mm}") for mm in range(4)]
                    if t + 2 < CAP:
                        load_x(t + 2)
                xt = pre_x[t]
                pq = ps_big.tile([128, S], f32, tag="big", name="big")
                for kc in range(2):
                    nc.tensor.matmul(
                        pq[:], wq_sb[:, kc, m * 128:(m + 1) * 128],
                        xt[:, kc, :], start=(kc == 0), stop=(kc == 1))
                nc.vector.tensor_copy(qT[t][m][:], pq[:])
                if m == 3:
                    pre_x.pop(t, None)

            def oproj_unit(t, outT_t, y4, lsub):
                po = ps_av.tile([128, DQ], f32, tag="av", name="av")
                for kc in range(4):
                    nc.tensor.matmul(
                        po[:], outT_t[kc][:, lsub * 128:(lsub + 1) * 128],
                        wo_sb[:, kc, :], start=(kc == 0), stop=(kc == 3))
                nc.vector.tensor_add(y4[:, lsub, :], po[:], bob_sb[:])
                if lsub == 3:
                    nc.sync.dma_start(
                        y[t * TL:(t + 1) * TL, :]
                        .rearrange("(l p) d -> p l d", p=128), y4[:])

            def emit_scores(t, s, h):
                c, half = h // 2, h % 2
                expT = []
                for g in range(2):
                    psc = ps_sc.tile([128, 2, TL], f32, tag="sc", name="sc")
                    for jj in range(2):
                        sc = g * 2 + jj
                        nc.tensor.matmul(
                            psc[:, jj, :],
                            kT[s][c][half * 64:(half + 1) * 64,
                                     sc * 128:(sc + 1) * 128],
                            qT[t][c][half * 64:(half + 1) * 64, :],
                            start=True, stop=True)
                    e = epool.tile([128, 2, TL], bf16, tag=f"e{g}",
                                   name=f"e{g}")
                    nc.scalar.activation(
                        e[:], psc[:], mybir.ActivationFunctionType.Exp)
                    expT.append(e)
                return expT

            # ---- prologue fast path: just enough for the first exp,
            # then the remaining slot-0 KV / tile-0 qproj units spread
            # across tile 0's head loop.
            kv_alloc(0)
            kproj(0, 0)
            qproj_unit(0, 0)
            pend_scores = emit_scores(0, 0, 0)
            vproj(0, 0)
            vproj(0, 1)
            kproj(0, 1)
            qproj_unit(0, 1)
            pending = [
                lambda: kproj(0, 2), lambda: kproj(0, 3),
                lambda: vproj(0, 2), lambda: vproj(0, 3),
                lambda: qproj_unit(0, 2), lambda: qproj_unit(0, 3),
            ]
            if NSLOT > 1:
                pending.append(lambda: load_ctx(1))

            slot_of = [max(j for j in range(NSLOT) if starts[j] <= t)
                       for t in range(CAP)]
            if CAP > 1 and slot_of[1] != 0:
                pending.extend(kv_units(1))

            for t in range(CAP):
                s = slot_of[t]
                if t + 1 < CAP:
                    # KV for a slot starting at t+2 is queued here (one
                    # tile early) so its units spread across two tiles
                    if t + 2 < CAP and slot_of[t + 2] != slot_of[t + 1]:
                        pending.extend(kv_units(slot_of[t + 2]))
                    pending.extend(
                        lambda tt=t + 1, m=m: qproj_unit(tt, m)
                        for m in range(4))
                outT = [opool.tile([128, TL], bf16, tag=f"o{m}",
                                   name=f"o{m}") for m in range(4)]
                y4 = ypool.tile([128, 4, DQ], f32, tag="y", name="y")
                last = t + 1 == CAP
                for h in range(H):
                    c, half = h // 2, h % 2
                    expT = pend_scores
                    if h + 1 < H:
                        pend_scores = emit_scores(t, s, h + 1)
                    elif not last:
                        pend_scores = emit_scores(t + 1, slot_of[t + 1], 0)
                    pav = ps_av.tile([VW, TL], f32, tag="av", name="av")
                    for g in range(2):
                        for jj in range(2):
                            sc = g * 2 + jj
                            nc.tensor.matmul(
                                pav[:], vA[s][sc][:, VW * h:VW * (h + 1)],
                                expT[g][:, jj, :],
                                start=(sc == 0), stop=(sc == 3))
                    rp = spool.tile([1, TL], f32, tag="rp", name="rp")
                    nc.vector.reciprocal(rp[:], pav[DH:DH + 1, :])
                    bc = spool.tile([64, TL], f32, tag="bc", name="bc")
                    nc.gpsimd.partition_broadcast(bc[:], rp[0:1, :])
                    nc.vector.tensor_mul(
                        outT[c][half * 64:(half + 1) * 64, :],
                        pav[0:DH, :], bc[:])
                    for _ in range(2):
                        if pending:
                            pending.pop(0)()
                while pending:
                    pending.pop(0)()
                for lsub in range(4):
                    if last:
                        oproj_unit(t, outT, y4, lsub)
                    else:
                        pending.append(
                            lambda tt=t, o=outT, yy=y4, l=lsub:
                            oproj_unit(tt, o, yy, l))
    nc.compile()
    return nc


def kernel(x, context, seq_lens, Wq, Wk, Wv, Wo, bo):
    from concourse.bass_utils import run_bass_kernel_spmd

    bf = ml_dtypes.bfloat16
    x = np.asarray(x, dtype=np.float32)
    context = np.asarray(context, dtype=np.float32)
    seq_lens = np.asarray(seq_lens, dtype=np.int32)
    Wq = np.asarray(Wq, dtype=np.float32)
    Wk = np.asarray(Wk, dtype=np.float32)
    Wv = np.asarray(Wv, dtype=np.float32)
    Wo = np.asarray(Wo, dtype=np.float32)
    bo = np.asarray(bo, dtype=np.float32)

    lens = np.clip(seq_lens, 1, L)
    nt = [int(math.ceil(int(n) / TL)) for n in lens]
    caps, cores = _plan(nt)
    CAP = sum(caps)
    NSLOT = len(caps)

    if caps not in _PROG_CACHE:
        _PROG_CACHE[caps] = _build_program(caps)
    nc = _PROG_CACHE[caps]

    scale = 1.0 / math.sqrt(DH)
    wq_in = (Wq * scale).astype(bf)
    wk_in = Wk.astype(bf)
    wv_in = Wv.astype(bf)
    wo_in = np.ascontiguousarray(Wo).astype(bf)
    bob_in = np.broadcast_to(bo[None, :], (128, DQ)).astype(np.float32).copy()
    vones_in = np.ones((128, 8, 1), dtype=bf)

    ctx_cache = {}

    def ctx_t(bi):
        if bi not in ctx_cache:
            ctx_cache[bi] = np.ascontiguousarray(context[bi].T).astype(bf)
        return ctx_cache[bi]

    in_maps = []
    for core in range(N_CORES):
        xt_core = np.zeros((CAP * TL, DQ), dtype=np.float32)
        m = {}
        for j in range(NSLOT):
            bi, t0 = cores[core][j]
            if bi >= 0:
                r0 = t0 * TL
                r1 = min(r0 + caps[j] * TL, L)
                if r1 > r0:
                    off = sum(caps[:j]) * TL
                    xt_core[off:off + (r1 - r0)] = x[bi, r0:r1]
                m[f"ctx{j}"] = ctx_t(bi)
            else:
                m[f"ctx{j}"] = ctx_t(0)
        m["xT"] = np.ascontiguousarray(xt_core.T).astype(bf)
        m["wq"] = wq_in
        m["wk"] = wk_in
        m["wv"] = wv_in
        m["wo"] = wo_in
        m["bob"] = bob_in
        m["vones"] = vones_in
        in_maps.append(m)

    res = run_bass_kernel_spmd(nc, in_maps, list(range(N_CORES)))

    out = np.zeros((B, L, DQ), dtype=np.float32)
    for core in range(N_CORES):
        yc = res.results[core]["y"]
        for j in range(NSLOT):
            bi, t0 = cores[core][j]
            if bi < 0:
                continue
            r0 = t0 * TL
            r1 = min(r0 + caps[j] * TL, int(lens[bi]))
            if r1 > r0:
                off = sum(caps[:j]) * TL
                out[bi, r0:r1] = yc[off:off + (r1 - r0)]
    return out


# revision 18
# speedup vs baseline: 1.1514x; 1.0644x over previous
"""CrossAttention Trainium2 kernel (bf16 matmuls, multi-slot ragged plan).

Reference computation (per batch b):
  q = x @ Wq; k = ctx @ Wk; v = ctx @ Wv   (multi-head, H=8, DH=64)
  out = softmax(q k^T / sqrt(DH)) v @ Wo + bo, rows >= seq_len zeroed.

Strategy: only rows < seq_len are computed ("ragged"); valid query tiles
(512 rows) are distributed across the 8 cores with a statically uniform
per-core structure: every core runs CAP = sum(caps) query tiles split into
up to 3 KV "slots" of caps[j] tiles each; slot j reads its own context
tensor. Which batch each slot holds is per-core DATA, so one SPMD program
serves all cores.

All matmuls run in bf16 (full PE rate; fp8 DoubleRow was tried and is 2x
faster on paper, but fp8's ~4% value noise blows the max-norm accuracy
gate on queries whose softmax concentrates on a single key).

On-chip layout is fully "transposed" (sequence on the free dim):
  xT [DQ, Lt] -> qT = Wq^T xT [INNER, Lt]
  kT = Wk^T ctxT [INNER, S];  v_aug = (ctx @ Wv | 1) per head [S, 8*65]
  scoresT_h [S, Lt] = kT_h^T qT_h  (per head, K=DH)
  expT = exp(scoresT) -> bf16  (no max subtraction: logits ~ N(0,1))
  pav [65, Lt] = v_aug_h^T expT_h  (ones column -> row 64 = denominator)
  per head: DVE recip -> Pool partition_broadcast -> DVE mul -> outT bf16
  out [Lt, DQ] = outT^T @ Wo + bo (bias added on DVE), y f32.

Emission spreads qproj/oproj/KV-projection work units across the head
loop so the in-order PE never clumps and starves the other engines; each
tensor is loaded with one merged DMA (the issuing sequencer serializes
DMA issues at ~650ns each), with the first KV/ctx loads issued on the
otherwise-idle Activation queue to shorten the prologue.
"""

import math
import sys

sys.path.insert(0, "/opt/trn_rl_repo")

import numpy as np
import ml_dtypes

B, L, S = 8, 8192, 512
DQ, DC = 256, 768
H, DH = 8, 64
INNER = H * DH
TL = 512          # query rows per tile
N_CORES = 8
VW = DH + 1       # v columns per head (64 v + ones)
WVN = H * VW      # 520


def _try_assign(nt, caps):
    """Per batch, pick piece counts k_j (caps[j]-sized pieces) covering
    nt[b] tiles, with per-slot totals <= N_CORES."""
    nslot = len(caps)
    opts = []
    for n in nt:
        o = []
        def gen(j, ks, cov):
            if j == nslot:
                if cov >= n:
                    o.append((sum(ks), tuple(ks)))
                return
            for k in range(0, N_CORES + 1):
                c = cov + k * caps[j]
                if c > n + caps[0] and k > 0:
                    break
                gen(j + 1, ks + [k], c)
        gen(0, [], 0)
        if not o:
            return None
        o.sort()
        opts.append([ks for _, ks in o])

    found = None
    used_acc = []

    def dfs(i, used):
        nonlocal found
        if found is not None:
            return
        if i == len(opts):
            found = list(used_acc)
            return
        for ks in opts[i]:
            nu = tuple(u + k for u, k in zip(used, ks))
            if all(u <= N_CORES for u in nu):
                used_acc.append(ks)
                dfs(i + 1, nu)
                used_acc.pop()
                if found is not None:
                    return

    dfs(0, tuple([0] * nslot))
    return found


def _plan(nt):
    """Choose caps (1-3 slots) and per-core piece placement. Returns
    (caps, cores) with cores[i][j] = (batch, tile0), batch -1 = pad."""
    best = None
    for cap_total in range(max(1, math.ceil(sum(nt) / N_CORES)),
                           max(nt) + 3):
        cands = []
        for c0 in range(cap_total, 0, -1):
            r = cap_total - c0
            if r == 0:
                cands.append((c0,))
                continue
            for c1 in range(min(c0, r), 0, -1):
                r2 = r - c1
                if r2 == 0:
                    cands.append((c0, c1))
                elif r2 <= c1:
                    cands.append((c0, c1, r2))
        for caps in sorted(cands, key=len):
            asg = _try_assign(nt, caps)
            if asg is not None:
                best = (caps, asg)
                break
        if best is not None:
            break
    assert best is not None
    caps, asg = best
    nslot = len(caps)
    pieces = [[] for _ in range(nslot)]
    for bi, ks in enumerate(asg):
        t0 = 0
        for j in range(nslot):
            for _ in range(ks[j]):
                pieces[j].append((bi, t0))
                t0 += caps[j]
    cores = []
    for i in range(N_CORES):
        cores.append([pieces[j][i] if i < len(pieces[j]) else (-1, 0)
                      for j in range(nslot)])
    return caps, cores


_PROG_CACHE = {}


def _build_program(caps):
    import concourse.mybir as mybir
    import concourse.tile as tile
    from concourse import bacc

    f32 = mybir.dt.float32
    bf16 = mybir.dt.bfloat16
    f8 = mybir.dt.float8e4
    DR = mybir.MatmulPerfMode.DoubleRow
    CAP = sum(caps)
    NSLOT = len(caps)
    starts = [sum(caps[:j]) for j in range(NSLOT)]

    nc = bacc.Bacc("TRN2", target_bir_lowering=False, debug=False,
                   num_devices=N_CORES)
    xTs = [nc.declare_dram_parameter(f"xT{r}", [128, 2, CAP * TL], f8,
                                     isOutput=False) for r in range(2)]
    ctxs = [[nc.declare_dram_parameter(f"ctx{j}_{r}", [384, 2 * S], f8,
                                       isOutput=False) for r in range(2)]
            for j in range(NSLOT)]
    wqs = [nc.declare_dram_parameter(f"wq{r}", [128, 2, INNER], f8,
                                     isOutput=False) for r in range(2)]
    wks = [nc.declare_dram_parameter(f"wk{r}", [384, 2 * INNER], f8,
                                     isOutput=False) for r in range(2)]
    wvs = [nc.declare_dram_parameter(f"wv{r}", [384, 2 * INNER], f8,
                                     isOutput=False) for r in range(2)]
    wo = nc.declare_dram_parameter("wo", [INNER, DQ], bf16, isOutput=False)
    bob = nc.declare_dram_parameter("bob", [128, DQ], f32, isOutput=False)
    vones = nc.declare_dram_parameter("vones", [128, 8, 1], bf16,
                                      isOutput=False)
    y = nc.declare_dram_parameter("y", [CAP * TL, DQ], f32, isOutput=True)

    with tile.TileContext(nc) as tc:
        with (
            tc.tile_pool(name="wpool", bufs=1) as wpool,
            tc.tile_pool(name="kvpool", bufs=1) as kvpool,
            tc.tile_pool(name="ctxpool", bufs=2) as ctxpool,
            tc.tile_pool(name="mpool", bufs=3) as mpool,
            tc.tile_pool(name="qpool", bufs=2) as qpool,
            tc.tile_pool(name="epool", bufs=5) as epool,
            tc.tile_pool(name="opool", bufs=2) as opool,
            tc.tile_pool(name="spool", bufs=6) as spool,
            tc.tile_pool(name="ypool", bufs=4) as ypool,
            tc.tile_pool(name="ps_big", bufs=2, space="PSUM") as ps_big,
            tc.tile_pool(name="ps_sc", bufs=2, space="PSUM") as ps_sc,
            tc.tile_pool(name="ps_av", bufs=2, space="PSUM") as ps_av,
        ):
            # ---- prologue loads; one merged DMA per tensor. ctx0/wk go on
            # the idle ACT queue so kproj can start ~2us in; the rest on SP.
            ctx_tiles = {}

            def load_ctx(j, eng=None):
                ts = []
                for r in range(2):
                    t = ctxpool.tile([128, 3, 2, S], f8, tag=f"ctx{r}",
                                     name=f"ctx{j}_{r}")
                    (eng or nc.sync).dma_start(
                        t[:], ctxs[j][r][:]
                        .rearrange("(kc p) (i s) -> p kc i s", p=128, i=2))
                    ts.append(t)
                ctx_tiles[j] = ts

            load_ctx(0)
            wk_sb = []
            for r in range(2):
                t = wpool.tile([128, 3, 2, INNER], f8, tag=f"wk{r}",
                               name=f"wk{r}")
                nc.scalar.dma_start(
                    t[:], wks[r][:]
                    .rearrange("(kc p) (i c) -> p kc i c", p=128, i=2))
                wk_sb.append(t)
            wq_sb = []
            for r in range(2):
                t = wpool.tile([128, 2, INNER], f8, tag=f"wq{r}",
                               name=f"wq{r}")
                nc.sync.dma_start(t[:], wqs[r][:])
                wq_sb.append(t)
            pre_x = {}

            def load_x(t):
                xts = []
                for r in range(2):
                    xt = mpool.tile([128, 2, TL], f8, tag=f"x{r}",
                                    name=f"x{t}_{r}")
                    nc.sync.dma_start(xt[:], xTs[r][:, :, t * TL:(t + 1) * TL])
                    xts.append(xt)
                pre_x[t] = xts

            load_x(0)
            wv_sb = []
            for r in range(2):
                t = wpool.tile([128, 3, 2, INNER], f8, tag=f"wv{r}",
                               name=f"wv{r}")
                nc.scalar.dma_start(
                    t[:], wvs[r][:]
                    .rearrange("(kc p) (i c) -> p kc i c", p=128, i=2))
                wv_sb.append(t)
            if CAP > 1:
                load_x(1)
            wo_sb = wpool.tile([128, 4, DQ], bf16, tag="wo", name="wo")
            nc.sync.dma_start(
                wo_sb[:], wo[:].rearrange("(kc p) d -> p kc d", p=128))
            bob_sb = wpool.tile([128, DQ], f32, tag="bob", name="bob")
            nc.sync.dma_start(bob_sb[:], bob[:])

            kT = {}
            vA = {}

            def kv_alloc(j):
                kT[j] = [kvpool.tile([128, S], bf16, tag=f"kT{j}_{m}",
                                     name=f"kT{j}_{m}") for m in range(4)]
                vA[j] = [kvpool.tile([128, WVN], bf16, tag=f"v{j}_{sc}",
                                     name=f"v{j}_{sc}") for sc in range(4)]

            PASSES = ((0, 0), (0, 1), (1, 0))  # (ctx/x residual, w residual)

            def kproj(j, m):
                ctx_sb = ctx_tiles[j]
                pk = ps_big.tile([128, S], f32, tag="big", name="big")
                first = True
                for ra, rb in PASSES:
                    for kc in range(3):
                        nc.tensor.matmul(
                            pk[:], wk_sb[rb][:, kc, :, m * 128:(m + 1) * 128],
                            ctx_sb[ra][:, kc, :, :], perf_mode=DR,
                            start=first,
                            stop=(ra, rb) == PASSES[-1] and kc == 2)
                        first = False
                nc.vector.tensor_copy(kT[j][m][:], pk[:])

            def vproj(j, sc):
                ctx_sb = ctx_tiles[j]
                pv = ps_big.tile([128, INNER], f32, tag="big", name="big")
                first = True
                for ra, rb in PASSES:
                    for kc in range(3):
                        nc.tensor.matmul(
                            pv[:],
                            ctx_sb[ra][:, kc, :, sc * 128:(sc + 1) * 128],
                            wv_sb[rb][:, kc, :, :], perf_mode=DR,
                            start=first,
                            stop=(ra, rb) == PASSES[-1] and kc == 2)
                        first = False
                vdst = vA[j][sc][:].rearrange("p (h d) -> p h d", d=VW)
                nc.vector.tensor_copy(
                    vdst[:, :, 0:DH],
                    pv[:].rearrange("p (h d) -> p h d", d=DH))
                nc.sync.dma_start(vdst[:, :, DH:DH + 1], vones[:])

            def kv_units(j):
                kv_alloc(j)
                units = [lambda m=m: kproj(j, m) for m in range(4)]
                units += [lambda sc=sc: vproj(j, sc) for sc in range(4)]
                if j + 1 < NSLOT:
                    units.append(lambda jj=j + 1: load_ctx(jj))
                return units

            qT = {}

            def qproj_unit(t, m):
                if m == 0:
                    qT[t] = [qpool.tile([128, TL], bf16, tag=f"q{mm}",
                                        name=f"q{mm}") for mm in range(4)]
                    if t + 2 < CAP:
                        load_x(t + 2)
                xt = pre_x[t]
                pq = ps_big.tile([128, S], f32, tag="big", name="big")
                for i, (ra, rb) in enumerate(PASSES):
                    nc.tensor.matmul(
                        pq[:], wq_sb[rb][:, :, m * 128:(m + 1) * 128],
                        xt[ra][:], perf_mode=DR,
                        start=(i == 0), stop=(i == 2))
                nc.vector.tensor_copy(qT[t][m][:], pq[:])
                if m == 3:
                    pre_x.pop(t, None)

            def oproj_unit(t, outT_t, y4, lsub):
                po = ps_av.tile([128, DQ], f32, tag="av", name="av")
                for kc in range(4):
                    nc.tensor.matmul(
                        po[:], outT_t[kc][:, lsub * 128:(lsub + 1) * 128],
                        wo_sb[:, kc, :], start=(kc == 0), stop=(kc == 3))
                nc.vector.tensor_add(y4[:, lsub, :], po[:], bob_sb[:])
                if lsub == 3:
                    nc.sync.dma_start(
                        y[t * TL:(t + 1) * TL, :]
                        .rearrange("(l p) d -> p l d", p=128), y4[:])

            def emit_scores(t, s, h):
                c, half = h // 2, h % 2
                expT = []
                for g in range(2):
                    psc = ps_sc.tile([128, 2, TL], f32, tag="sc", name="sc")
                    for jj in range(2):
                        sc = g * 2 + jj
                        nc.tensor.matmul(
                            psc[:, jj, :],
                            kT[s][c][half * 64:(half + 1) * 64,
                                     sc * 128:(sc + 1) * 128],
                            qT[t][c][half * 64:(half + 1) * 64, :],
                            start=True, stop=True)
                    e = epool.tile([128, 2, TL], bf16, tag=f"e{g}",
                                   name=f"e{g}")
                    nc.scalar.activation(
                        e[:], psc[:], mybir.ActivationFunctionType.Exp,
                        scale=1.0 / 512.0)
                    expT.append(e)
                return expT

            # ---- prologue fast path: just enough for the first exp,
            # then the remaining slot-0 KV / tile-0 qproj units spread
            # across tile 0's head loop.
            kv_alloc(0)
            kproj(0, 0)
            qproj_unit(0, 0)
            pend_scores = emit_scores(0, 0, 0)
            vproj(0, 0)
            vproj(0, 1)
            kproj(0, 1)
            qproj_unit(0, 1)
            pending = [
                lambda: kproj(0, 2), lambda: kproj(0, 3),
                lambda: vproj(0, 2), lambda: vproj(0, 3),
                lambda: qproj_unit(0, 2), lambda: qproj_unit(0, 3),
            ]
            if NSLOT > 1:
                pending.append(lambda: load_ctx(1))

            slot_of = [max(j for j in range(NSLOT) if starts[j] <= t)
                       for t in range(CAP)]
            if CAP > 1 and slot_of[1] != 0:
                pending.extend(kv_units(1))

            for t in range(CAP):
                s = slot_of[t]
                if t + 1 < CAP:
                    # KV for a slot starting at t+2 is queued here (one
                    # tile early) so its units spread across two tiles
                    if t + 2 < CAP and slot_of[t + 2] != slot_of[t + 1]:
                        pending.extend(kv_units(slot_of[t + 2]))
                    pending.extend(
                        lambda tt=t + 1, m=m: qproj_unit(tt, m)
                        for m in range(4))
                outT = [opool.tile([128, TL], bf16, tag=f"o{m}",
                                   name=f"o{m}") for m in range(4)]
                y4 = ypool.tile([128, 4, DQ], f32, tag="y", name="y")
                last = t + 1 == CAP
                for h in range(H):
                    c, half = h // 2, h % 2
                    expT = pend_scores
                    if h + 1 < H:
                        pend_scores = emit_scores(t, s, h + 1)
                    elif not last:
                        pend_scores = emit_scores(t + 1, slot_of[t + 1], 0)
                    pav = ps_av.tile([VW, TL], f32, tag="av", name="av")
                    for g in range(2):
                        for jj in range(2):
                            sc = g * 2 + jj
                            nc.tensor.matmul(
                                pav[:], vA[s][sc][:, VW * h:VW * (h + 1)],
                                expT[g][:, jj, :],
                                start=(sc == 0), stop=(sc == 3))
                    rp = spool.tile([1, TL], f32, tag="rp", name="rp")
                    nc.vector.reciprocal(rp[:], pav[DH:DH + 1, :])
                    bc = spool.tile([64, TL], f32, tag="bc", name="bc")
                    nc.gpsimd.partition_broadcast(bc[:], rp[0:1, :])
                    nc.vector.tensor_mul(
                        outT[c][half * 64:(half + 1) * 64, :],
                        pav[0:DH, :], bc[:])
                    for _ in range(2):
                        if pending:
                            pending.pop(0)()
                while pending:
                    pending.pop(0)()
                for lsub in range(4):
                    if last:
                        oproj_unit(t, outT, y4, lsub)
                    else:
                        pending.append(
                            lambda tt=t, o=outT, yy=y4, l=lsub:
                            oproj_unit(tt, o, yy, l))
    nc.compile()
    return nc


def kernel(x, context, seq_lens, Wq, Wk, Wv, Wo, bo):
    from concourse.bass_utils import run_bass_kernel_spmd

    bf = ml_dtypes.bfloat16
    x = np.asarray(x, dtype=np.float32)
    context = np.asarray(context, dtype=np.float32)
    seq_lens = np.asarray(seq_lens, dtype=np.int32)
    Wq = np.asarray(Wq, dtype=np.float32)
    Wk = np.asarray(Wk, dtype=np.float32)
    Wv = np.asarray(Wv, dtype=np.float32)
    Wo = np.asarray(Wo, dtype=np.float32)
    bo = np.asarray(bo, dtype=np.float32)

    lens = np.clip(seq_lens, 1, L)
    nt = [int(math.ceil(int(n) / TL)) for n in lens]
    caps, cores = _plan(nt)
    CAP = sum(caps)
    NSLOT = len(caps)

    if caps not in _PROG_CACHE:
        _PROG_CACHE[caps] = _build_program(caps)
    nc = _PROG_CACHE[caps]

    f8 = ml_dtypes.float8_e4m3fn

    def f8pair(a):
        hi = a.astype(f8)
        lo = (a - hi.astype(np.float32)).astype(f8)
        return hi, lo

    def dr_layout(w):
        dc, x_ = w.shape
        return np.ascontiguousarray(
            w.reshape(3, 2, 128, x_).transpose(0, 2, 1, 3)
            .reshape(384, 2 * x_))

    # fp8 e4m3's min normal is 2^-6; W entries are ~1/sqrt(fan-in), so
    # upscale x8 before quantizing (descaled via the exp scale / vones)
    wq_hi, wq_lo = f8pair(np.ascontiguousarray(
        (Wq * 8.0).reshape(2, 128, INNER).transpose(1, 0, 2)))
    wk_hi, wk_lo = f8pair(dr_layout(Wk * 8.0))
    wv_hi, wv_lo = f8pair(dr_layout(Wv * 8.0))
    wo_in = np.ascontiguousarray(Wo).astype(bf)
    bob_in = np.broadcast_to(bo[None, :], (128, DQ)).astype(np.float32).copy()
    vones_in = np.full((128, 8, 1), 8.0, dtype=bf)

    ctx_cache = {}

    def ctx_t(bi):
        if bi not in ctx_cache:
            ctx_cache[bi] = f8pair(
                dr_layout(np.ascontiguousarray(context[bi].T)))
        return ctx_cache[bi]

    in_maps = []
    for core in range(N_CORES):
        xt_core = np.zeros((CAP * TL, DQ), dtype=np.float32)
        m = {}
        for j in range(NSLOT):
            bi, t0 = cores[core][j]
            if bi >= 0:
                r0 = t0 * TL
                r1 = min(r0 + caps[j] * TL, L)
                if r1 > r0:
                    off = sum(caps[:j]) * TL
                    xt_core[off:off + (r1 - r0)] = x[bi, r0:r1]
                ch, cl = ctx_t(bi)
            else:
                ch, cl = ctx_t(0)
            m[f"ctx{j}_0"], m[f"ctx{j}_1"] = ch, cl
        xh, xl = f8pair(np.ascontiguousarray(
            xt_core.T.reshape(2, 128, CAP * TL).transpose(1, 0, 2)))
        m["xT0"], m["xT1"] = xh, xl
        m["wq0"], m["wq1"] = wq_hi, wq_lo
        m["wk0"], m["wk1"] = wk_hi, wk_lo
        m["wv0"], m["wv1"] = wv_hi, wv_lo
        m["wo"] = wo_in
        m["bob"] = bob_in
        m["vones"] = vones_in
        in_maps.append(m)

    res = run_bass_kernel_spmd(nc, in_maps, list(range(N_CORES)))

    out = np.zeros((B, L, DQ), dtype=np.float32)
    for core in range(N_CORES):
        yc = res.results[core]["y"]
        for j in range(NSLOT):
            bi, t0 = cores[core][j]
            if bi < 0:
                continue
            r0 = t0 * TL
            r1 = min(r0 + caps[j] * TL, int(lens[bi]))
            if r1 > r0:
                off = sum(caps[:j]) * TL
                out[bi, r0:r1] = yc[off:off + (r1 - r0)]
    return out
